# revision 1
# baseline (speedup 1.0000x reference)
"""GQA attention with 2D RoPE on 8 TRN2 NeuronCores.

Sharding: batch data-parallel x4  X  head-group tensor-parallel x2.
Core c handles batch b=c//2 and head group g=c%2 (16 Q heads, 4 KV heads).
wo is row-sharded; partials are ReduceScattered across each core pair
(f32 wire, two [896, 1024] chunks), so core 2b returns rows 0:448 and core
2b+1 rows 448:896 of batch b.

Layouts (everything "transposed" so contraction dims sit on partitions):
  xT   [D, L]      QT/KT  [o, L]   (head-dim rows, RoPE'd, bf16)
  V    [L, o_v]    Vext   per (kv head, key block): [128, 128] = [V|1] or [1|V]
  S^T  [keys, q]   U = exp(S^T/8) masked, bf16
  O^T  [d, q] accumulated in PSUM via lhsT=Vext
  aT   [i, L] bf16 -> out = aT.T @ woT_g  (psum [l, o]) -> AllReduce
"""

import math
import numpy as np

import concourse.bass as bass
import concourse.tile as tile
import concourse.mybir as mybir
from concourse import bacc
from concourse import bass_utils

F32 = mybir.dt.float32
F32R = mybir.dt.float32r
BF16 = mybir.dt.bfloat16
I32 = mybir.dt.int32
AF = mybir.ActivationFunctionType
ALU = mybir.AluOpType

B, L, D = 4, 896, 2048
HQ, HKV, HD = 32, 8, 64
NCORES = 8
GO = D // 2          # 1024 q-out dims per core
KVO = HKV * HD // 2  # 256 kv-out dims per core
NH = 16              # q heads per core
NKV = 4              # kv heads per core
P = 128
NI = D // P          # 16 contraction chunks
LB = L // P          # 7 key/l blocks
QCN = 4              # q chunks
QCW = L // QCN       # 224 q-chunk width
NKB = [2, 4, 6, 7]   # key blocks per q chunk (causal)
ROPE_BASE = 10000.0
TWO_PI = 2.0 * math.pi

# (qc, kb) pairs needing a causal mask, with affine_select base = 224*qc - 128*kb
PARTIAL = {}
for _qc in range(QCN):
    for _kb in range(NKB[_qc]):
        lo_key, hi_key = 128 * _kb, 128 * _kb + 127
        lo_row, hi_row = QCW * _qc, QCW * (_qc + 1) - 1
        if hi_key > lo_row:  # some key exceeds some row -> partial
            PARTIAL[(_qc, _kb)] = QCW * _qc - 128 * _kb

_NC_CACHE = {}


def build_nc(with_collective=True):
    key = with_collective
    if key in _NC_CACHE:
        return _NC_CACHE[key]
    nc = bacc.Bacc("TRN2", target_bir_lowering=False, debug=False,
                   num_devices=NCORES)
    ins = {
        "xT": nc.dram_tensor("xT", [D, L], F32R, kind="ExternalInput").ap(),
        "wqT": nc.dram_tensor("wqT", [D, GO], F32R, kind="ExternalInput").ap(),
        "wkT": nc.dram_tensor("wkT", [D, KVO], F32R, kind="ExternalInput").ap(),
        "wvT": nc.dram_tensor("wvT", [D, KVO], F32R, kind="ExternalInput").ap(),
        "woT": nc.dram_tensor("woT", [GO, D], F32, kind="ExternalInput").ap(),
        "pos_t": nc.dram_tensor("pos_t", [L], F32, kind="ExternalInput").ap(),
        "pos_s": nc.dram_tensor("pos_s", [L], F32, kind="ExternalInput").ap(),
    }
    y = nc.dram_tensor("y", [L // 2, D], F32, kind="ExternalOutput").ap()
    with tile.TileContext(nc) as tc:
        _build_kernel(nc, tc, ins, y, with_collective)
    nc.compile()
    _NC_CACHE[key] = nc
    return nc


def _bcast_row(dram_ap, parts, n):
    """AP reading a [n] dram tensor broadcast to `parts` partitions."""
    return bass.AP(tensor=dram_ap.tensor, offset=dram_ap.offset,
                   ap=[[0, parts], [1, n]])


def _build_tables(nc, tc, ctx, ins, const):
    """RoPE cos/sin maps C,S [128, L] bf16 + 10 causal masks [128,224] bf16."""
    with tc.tile_pool(name="tbl_tmp", bufs=10) as tmp:
        _build_tables_inner(nc, tc, tmp, ins, const)
    masks = {}
    for (qc, kb), base in PARTIAL.items():
        m = const.tile([P, QCW], BF16, tag=f"mask{qc}_{kb}", name=f"mask{qc}_{kb}")
        nc.vector.memset(m[:], 1.0)
        nc.gpsimd.affine_select(out=m[:], in_=m[:], compare_op=ALU.is_ge,
                                fill=0.0, base=base, channel_multiplier=-1,
                                pattern=[[1, QCW]])
        masks[(qc, kb)] = m
    C, S = _build_tables.CS
    return C, S, masks


def _build_tables_inner(nc, tc, tmp, ins, const):
    it = tmp.tile([16, 1], I32)
    nc.gpsimd.iota(it[:], pattern=[[0, 1]], base=0, channel_multiplier=1)
    itf = tmp.tile([16, 1], F32)
    nc.vector.tensor_copy(itf[:], it[:])
    invf = tmp.tile([16, 1], F32)
    nc.scalar.activation(invf[:], itf[:], AF.Exp,
                         scale=-math.log(ROPE_BASE) / 16.0)

    C = const.tile([P, L], BF16)
    S = const.tile([P, L], BF16)

    for name, which in (("pos_t", 0), ("pos_s", 1)):
        posb = tmp.tile([16, L], F32, tag="tt")
        nc.sync.dma_start(posb[:], _bcast_row(ins[name], 16, L))
        freq = tmp.tile([16, L], F32, tag="tt")
        nc.vector.tensor_scalar_mul(freq[:], posb[:], invf[:])
        for trig in (0, 1):  # 0 -> cos (shift +pi/2 before reduction), 1 -> sin
            shifted = tmp.tile([16, L], F32, tag="tt")
            if trig == 0:
                nc.vector.tensor_scalar_add(shifted[:], freq[:], math.pi / 2)
            else:
                nc.vector.tensor_copy(shifted[:], freq[:])
            g = tmp.tile([16, L], F32, tag="tt")
            nc.vector.tensor_scalar_mul(g[:], shifted[:], 1.0 / TWO_PI)
            gi = tmp.tile([16, L], I32, tag="tt")
            nc.vector.tensor_copy(gi[:], g[:])  # trunc (sim) / rint (hw)
            gf = tmp.tile([16, L], F32, tag="tt")
            nc.vector.tensor_copy(gf[:], gi[:])
            nc.vector.tensor_scalar_mul(gf[:], gf[:], TWO_PI)
            red = tmp.tile([16, L], F32, tag="tt")
            nc.vector.tensor_sub(red[:], shifted[:], gf[:])
            # fold into [-pi, pi] (robust to either cast rounding mode):
            # red > pi -> -= 2pi ; red < -pi -> += 2pi
            for cmp_op, sign, thr in ((ALU.is_gt, -TWO_PI, math.pi),
                                      (ALU.is_lt, TWO_PI, -math.pi)):
                cm = tmp.tile([16, L], F32, tag="tt")
                nc.vector.tensor_scalar(cm[:], red[:], thr, sign,
                                        op0=cmp_op, op1=ALU.mult)
                nc.vector.tensor_add(red[:], red[:], cm[:])
            nc.vector.tensor_scalar_min(red[:], red[:], math.pi)
            nc.vector.tensor_scalar_max(red[:], red[:], -math.pi)
            resf = tmp.tile([16, L], F32, tag="tt")
            nc.scalar.activation(resf[:], red[:], AF.Sin)
            res = tmp.tile([16, L], BF16, tag="tt")
            nc.vector.tensor_copy(res[:], resf[:])
            # scatter into C/S row slots via DMA (cross-partition writes)
            # 64-row pattern: [cos_t, cos_t, cos_s, cos_s] ; S: [-st, st, -ss, ss]
            if trig == 0:
                dsts = [(C, 0), (C, 16)] if which == 0 else [(C, 32), (C, 48)]
                for dst, off in dsts:
                    for rep in (0, 64):
                        nc.sync.dma_start(dst[off + rep:off + rep + 16, :], res[:])
            else:
                neg = tmp.tile([16, L], BF16, tag="tt")
                nc.vector.tensor_scalar_mul(neg[:], resf[:], -1.0)
                base = 0 if which == 0 else 32
                for rep in (0, 64):
                    nc.sync.dma_start(S[base + rep:base + rep + 16, :], neg[:])
                    nc.sync.dma_start(S[base + rep + 16:base + rep + 32, :], res[:])

    _build_tables.CS = (C, S)


def _rope(nc, tc, pool, raw, C, S, out):
    """out = raw*C + shuffle16(raw)*S   (all [128, L] bf16)."""
    shuf = pool.tile([P, L], BF16, tag="rope_shuf")
    mask = [(p ^ 16) for p in range(32)]
    nc.vector.stream_shuffle(shuf[:], raw[:], mask)
    m1 = pool.tile([P, L], BF16, tag="rope_m1")
    nc.vector.tensor_mul(m1[:], raw[:], C[:])
    m2 = pool.tile([P, L], BF16, tag="rope_m2")
    nc.vector.tensor_mul(m2[:], shuf[:], S[:])
    nc.vector.tensor_add(out[:], m1[:], m2[:])


def _build_kernel(nc, tc, ins, y, with_collective):
    import contextlib
    ctx = contextlib.ExitStack()
    with ctx:
        const = ctx.enter_context(tc.tile_pool(name="const", bufs=1))

        # ---------------- persistent activation storage ----------------
        qt_pool = ctx.enter_context(tc.tile_pool(name="qt", bufs=1))
        kt_pool = ctx.enter_context(tc.tile_pool(name="kt", bufs=1))
        v_pool = ctx.enter_context(tc.tile_pool(name="vx", bufs=1))
        at_pool = ctx.enter_context(tc.tile_pool(name="at", bufs=1))
        QT = [qt_pool.tile([P, L], BF16, tag=f"qt{i}", name=f"qt{i}") for i in range(8)]
        KTd = [kt_pool.tile([P, L], BF16, tag=f"kt{i}", name=f"kt{i}") for i in range(NKV)]
        # Vext[kv][kb][variant]: variant 0 = [V|1], 1 = [1|V]
        Vext = [[[v_pool.tile([P, P], BF16, tag=f"v{k}_{b_}_{vr}", name=f"v{k}_{b_}_{vr}")
                  for vr in range(2)] for b_ in range(LB)] for k in range(NKV)]
        AT = [at_pool.tile([P, L], BF16, tag=f"at{i}", name=f"at{i}") for i in range(8)]
        for k in range(NKV):
            for b_ in range(LB):
                nc.vector.memset(Vext[k][b_][0][:, 64:128], 1.0)
                nc.vector.memset(Vext[k][b_][1][:, 0:64], 1.0)

        # ---------------- phase 1: projections + rope -------------------
        with tc.tile_pool(name="xt", bufs=1) as xt_pool, \
             tc.tile_pool(name="wst", bufs=4) as wst, \
             tc.tile_pool(name="ev", bufs=4) as ev, \
             tc.tile_pool(name="ps1", bufs=1, space="PSUM") as ps1:
            XT = [xt_pool.tile([P, L], F32R, tag=f"xt{i}", name=f"xt{i}") for i in range(NI)]

            # V: out[l, o_v] ; lhsT = xT chunk slice, rhs = wvT chunk
            psv = [ps1.tile([P, KVO], F32, tag=f"ps1_{b_}", name=f"psv{b_}") for b_ in range(LB)]
            for i in range(NI):
                nc.sync.dma_start(XT[i][:], ins["xT"][i * P:(i + 1) * P, :])
                wv = wst.tile([P, KVO], F32R, tag="wv")
                nc.sync.dma_start(wv[:], ins["wvT"][i * P:(i + 1) * P, :])
                for b_ in range(LB):
                    nc.tensor.matmul(
                        psv[b_][:], XT[i][:, b_ * P:(b_ + 1) * P],
                        wv[:], start=(i == 0), stop=(i == NI - 1))
            for b_ in range(LB):
                for k in range(NKV):
                    sl = psv[b_][:, k * 64:(k + 1) * 64]
                    nc.scalar.copy(Vext[k][b_][0][:, 0:64], sl)
                    nc.scalar.copy(Vext[k][b_][1][:, 64:128], sl)

            # K: KT[o, l] ; lhsT = wkT chunk slice, rhs = xT chunk
            psk = [ps1.tile([P, 448], F32, tag=f"ps1_{j}", name=f"psk{j}") for j in range(4)]
            for i in range(NI):
                wk = wst.tile([P, KVO], F32R, tag="wk")
                nc.sync.dma_start(wk[:], ins["wkT"][i * P:(i + 1) * P, :])
                for ob in range(2):
                    for h2 in range(2):
                        nc.tensor.matmul(
                            psk[ob * 2 + h2][:],
                            wk[:, ob * P:(ob + 1) * P],
                            XT[i][:, h2 * 448:(h2 + 1) * 448],
                            start=(i == 0), stop=(i == NI - 1))
            C, S, masks = _build_tables(nc, tc, ctx, ins, const)
            for ob in range(2):
                raw = ev.tile([P, L], BF16, tag="kraw")
                for h2 in range(2):
                    nc.vector.tensor_copy(raw[:, h2 * 448:(h2 + 1) * 448],
                                          psk[ob * 2 + h2][:])
                roped = ev.tile([P, L], BF16, tag="kroped")
                _rope(nc, tc, ev, raw, C, S, roped)
                # duplicate each kv head across both partition halves
                for sub in range(2):
                    k = ob * 2 + sub
                    src = roped[sub * 64:(sub + 1) * 64, :]
                    nc.sync.dma_start(KTd[k][0:64, :], src)
                    nc.sync.dma_start(KTd[k][64:128, :], src)

            # Q: QT[o, l] ; two groups of 4 ob-blocks (8 psums each)
            for og in range(2):
                psq = [ps1.tile([P, 448], F32, tag=f"ps1_{j}", name=f"psq{j}") for j in range(8)]
                for i in range(NI):
                    wq = wst.tile([P, 512], F32R, tag="wq")
                    nc.sync.dma_start(
                        wq[:], ins["wqT"][i * P:(i + 1) * P,
                                          og * 512:(og + 1) * 512])
                    for ob in range(4):
                        for h2 in range(2):
                            nc.tensor.matmul(
                                psq[ob * 2 + h2][:],
                                wq[:, ob * P:(ob + 1) * P],
                                XT[i][:, h2 * 448:(h2 + 1) * 448],
                                start=(i == 0), stop=(i == NI - 1))
                for ob in range(4):
                    raw = ev.tile([P, L], BF16, tag="qraw")
                    for h2 in range(2):
                        nc.vector.tensor_copy(raw[:, h2 * 448:(h2 + 1) * 448],
                                              psq[ob * 2 + h2][:])
                    _rope(nc, tc, ev, raw, C, S, QT[og * 4 + ob])

        # ---------------- phase 2: attention ----------------------------
        with tc.tile_pool(name="uatt", bufs=6) as upool, \
             tc.tile_pool(name="rec", bufs=6) as recpool, \
             tc.tile_pool(name="pss", bufs=2, space="PSUM") as pss, \
             tc.tile_pool(name="psav", bufs=4, space="PSUM") as psav:
            for h in range(NH):
                kv = h // 4
                qblk, qsub = divmod(h, 2)
                qoff = qsub * 64
                soff = 64 - qoff
                vr = qsub
                for qc in range(QCN):
                    nkb = NKB[qc]
                    qsl = slice(qc * QCW, (qc + 1) * QCW)
                    ps_av = psav.tile([P, QCW], F32, tag="av",
                                      name=f"av{h}_{qc}")
                    kb = 0
                    for k0 in range(0, nkb, 4):
                        ng = min(4, nkb - k0)
                        ps_s = pss.tile([P, 4, 256], F32, tag="s",
                                        name=f"s{h}_{qc}_{k0}")
                        for j in range(ng):
                            nc.tensor.matmul(
                                ps_s[:, j, 0:QCW],
                                KTd[kv][qoff:qoff + 64,
                                        (k0 + j) * P:(k0 + j + 1) * P],
                                QT[qblk][qoff:qoff + 64, qsl],
                                start=True, stop=True,
                                tile_position=(qoff, 0))
                        U = upool.tile([P, 4, 256], BF16, tag="u",
                                       name=f"u{h}_{qc}_{k0}")
                        nc.scalar.activation(U[:, 0:ng, 0:QCW],
                                             ps_s[:, 0:ng, 0:QCW],
                                             AF.Exp, scale=0.125)
                        for j in range(ng):
                            if (qc, k0 + j) in PARTIAL:
                                nc.gpsimd.tensor_tensor(
                                    U[:, j, 0:QCW], U[:, j, 0:QCW],
                                    masks[(qc, k0 + j)][:], op=ALU.mult)
                            nc.tensor.matmul(
                                ps_av[:], Vext[kv][k0 + j][vr][:],
                                U[:, j, 0:QCW],
                                start=(kb == 0), stop=(kb == nkb - 1))
                            kb += 1
                    recs = recpool.tile([P, QCW], F32, tag="recs",
                                        name=f"recs{h}_{qc}")
                    nc.vector.reciprocal(recs[soff:soff + 64, :],
                                         ps_av[soff:soff + 64, :])
                    rec = recpool.tile([P, QCW], F32, tag="rec",
                                       name=f"rec{h}_{qc}")
                    nc.sync.dma_start(rec[qoff:qoff + 64, :],
                                      recs[soff:soff + 64, :])
                    nc.vector.tensor_mul(AT[qblk][qoff:qoff + 64, qsl],
                                         ps_av[qoff:qoff + 64, :],
                                         rec[qoff:qoff + 64, :])

        # ---------------- phase 3: out projection + reduce-scatter -------
        with tc.tile_pool(name="wo", bufs=1) as wopool, \
             tc.tile_pool(name="wof", bufs=4) as wofpool, \
             tc.tile_pool(name="osb", bufs=6) as osb, \
             tc.tile_pool(name="pso", bufs=1, space="PSUM") as pso, \
             tc.tile_pool(name="ccdram", bufs=1, space="DRAM") as ccdram:
            WOB = {}
            for oc in range(4):
                for ic in range(8):
                    wof = wofpool.tile([P, 512], F32, tag="wof",
                                       name=f"wof{oc}_{ic}")
                    nc.sync.dma_start(
                        wof[:], ins["woT"][ic * P:(ic + 1) * P,
                                           oc * 512:(oc + 1) * 512])
                    wob = wopool.tile([P, 512], BF16, tag=f"wob{oc}_{ic}",
                                      name=f"wob{oc}_{ic}")
                    nc.scalar.copy(wob[:], wof[:])
                    WOB[(oc, ic)] = wob
            cc_in = [ccdram.tile([L, 1024], BF16, tag=f"ccin{g_}", name=f"ccin{g_}")
                     for g_ in range(2)]
            cc_out = [ccdram.tile([L // 2, 1024], BF16, tag=f"ccout{g_}", name=f"ccout{g_}")
                      for g_ in range(2)]
            for oc in range(4):
                pso_t = [pso.tile([P, 512], F32, tag=f"pso{b_}", name=f"pso{oc}_{b_}")
                         for b_ in range(LB)]
                for ic in range(8):
                    for b_ in range(LB):
                        nc.tensor.matmul(pso_t[b_][:],
                                         AT[ic][:, b_ * P:(b_ + 1) * P],
                                         WOB[(oc, ic)][:], start=(ic == 0),
                                         stop=(ic == 7))
                g_, half = divmod(oc, 2)
                for b_ in range(LB):
                    ot = osb.tile([P, 512], BF16, tag="ot", name=f"ot{oc}_{b_}")
                    nc.vector.tensor_copy(ot[:], pso_t[b_][:])
                    nc.sync.dma_start(
                        cc_in[g_][b_ * P:(b_ + 1) * P,
                                  half * 512:(half + 1) * 512], ot[:])
                if half == 1:
                    src_dram = cc_out[g_]
                    if with_collective:
                        nc.gpsimd.collective_compute(
                            "ReduceScatter", ALU.add,
                            replica_groups=[[0, 1], [2, 3], [4, 5], [6, 7]],
                            ins=[cc_in[g_].opt()], outs=[cc_out[g_].opt()])
                    else:
                        src_dram = cc_in[g_]
                    # bf16 -> f32 via SBUF bounce (no casting DMAs)
                    for r0, rn in ((0, P), (P, P), (2 * P, P), (3 * P, 64)):
                        yb = osb.tile([P, 1024], BF16, tag="yb",
                                      name=f"yb{g_}_{r0}")
                        nc.sync.dma_start(yb[0:rn, :],
                                          src_dram[r0:r0 + rn, :])
                        yf = osb.tile([P, 1024], F32, tag="yf",
                                      name=f"yf{g_}_{r0}")
                        nc.vector.tensor_copy(yf[0:rn, :], yb[0:rn, :])
                        nc.sync.dma_start(
                            y[r0:r0 + rn, g_ * 1024:(g_ + 1) * 1024],
                            yf[0:rn, :])


# ---------------------------------------------------------------- host side
_ROPE_PERM = np.concatenate([
    np.arange(0, 32, 2), np.arange(1, 32, 2),
    np.arange(32, 64, 2), np.arange(33, 64, 2)])


def make_in_maps(x, wq, wk, wv, wo, temporal_pos, structural_pos):
    x = np.asarray(x, dtype=np.float32)
    wq = np.asarray(wq, dtype=np.float32)
    wk = np.asarray(wk, dtype=np.float32)
    wv = np.asarray(wv, dtype=np.float32)
    wo = np.asarray(wo, dtype=np.float32)
    pt = np.ascontiguousarray(np.asarray(temporal_pos).astype(np.float32))
    ps = np.ascontiguousarray(np.asarray(structural_pos).astype(np.float32))

    wq_p = wq.reshape(HQ, HD, D)[:, _ROPE_PERM, :].reshape(D, D)
    wk_p = wk.reshape(HKV, HD, D)[:, _ROPE_PERM, :].reshape(HKV * HD, D)
    wqT = np.ascontiguousarray(wq_p.T)   # [D, D]
    wkT = np.ascontiguousarray(wk_p.T)   # [D, 512]
    wvT = np.ascontiguousarray(wv.T)     # [D, 512]
    woT = np.ascontiguousarray(wo.T)     # [D, D]

    in_maps = []
    for c in range(NCORES):
        b, g = divmod(c, 2)
        in_maps.append({
            "xT": np.ascontiguousarray(x[b].T),
            "wqT": np.ascontiguousarray(wqT[:, g * GO:(g + 1) * GO]),
            "wkT": np.ascontiguousarray(wkT[:, g * KVO:(g + 1) * KVO]),
            "wvT": np.ascontiguousarray(wvT[:, g * KVO:(g + 1) * KVO]),
            "woT": np.ascontiguousarray(woT[g * GO:(g + 1) * GO, :]),
            "pos_t": pt,
            "pos_s": ps,
        })
    return in_maps


def kernel(x, wq, wk, wv, wo, temporal_pos, structural_pos, _trace=False):
    nc = build_nc(with_collective=True)
    in_maps = make_in_maps(x, wq, wk, wv, wo, temporal_pos, structural_pos)
    res = bass_utils.run_bass_kernel_spmd(
        nc, in_maps, core_ids=list(range(NCORES)), trace=_trace)
    out = np.stack([
        np.concatenate([res.results[2 * b]["y"], res.results[2 * b + 1]["y"]],
                       axis=0) for b in range(B)])
    kernel.last_result = res
    return out.astype(np.float32)



# revision 23
# speedup vs baseline: 1.9730x; 1.9730x over previous
"""GQA attention with 2D RoPE on 8 TRN2 NeuronCores.

Sharding: batch data-parallel x4  X  head-group tensor-parallel x2.
Core c handles batch b=c//2 and head group g=c%2 (16 Q heads, 4 KV heads).
Each core writes its PARTIAL out-projection y_g = A_g @ woT_g  [896, 2048]
(bf16); the host sums the two partials per batch (no device collective).

All weights/activations enter as bf16 (host-cast).  RoPE cos/sin tables are
host-computed.  Q heads are host-permuted so the 4 Q heads of each KV head
live in the same partition half as that KV head (no K duplication): QT tile
t, slot s  <->  local q head  8*(t//4) + 4*s + t%4, kv head 2*(t//4)+s.

Attention uses 128-wide q blocks, 28 causal (qb, kb) blocks per head in
qb-major order, batched 4-at-a-time into one PSUM bank so exp runs as 7
Activation instructions per head.  The two slots of each QT tile run as two
interleaved streams so the PE works one stream while the other waits on exp;
the og1 (tiles 4..7) Q projection is drained between attention batches as
PE filler.

V staging per key block kb: Vst[kb] [128 keys, 9, 64] = [1|V0|1|V1|...|V3|1]
(64-col groups).  AV lhsT for kv head k, slot s = cols [128k + 64*(1-s),
+128) which is [V_k|1] for s=0 (numerators rows 0:64, denominator sums rows
64:128) and [1|V_k] for s=1.
"""

import math
import numpy as np
import ml_dtypes

import concourse.bass as bass
import concourse.tile as tile
import concourse.mybir as mybir
from concourse import bacc
from concourse import bass_utils

F32 = mybir.dt.float32
BF16 = mybir.dt.bfloat16
F8 = mybir.dt.float8e4
AF = mybir.ActivationFunctionType
ALU = mybir.AluOpType

B, L, D = 4, 896, 2048
HQ, HKV, HD = 32, 8, 64
NCORES = 8
GO = D // 2          # 1024 q-out dims per core
KVO = HKV * HD // 2  # 256 kv-out dims per core
P = 128
NI = D // P          # 16 contraction chunks
LB = L // P          # 7 key/q blocks

# causal blocks per head, qb-major: [(qb, kb)], kb <= qb
BLOCKS = [(qb, kb) for qb in range(LB) for kb in range(qb + 1)]
SX, SW = 8.0, 64.0   # fp8 operand scales (psum carries SX*SW = 512x)
EXPB = 4             # blocks per exp batch
NBATCH = (len(BLOCKS) + EXPB - 1) // EXPB   # 7 batches of 4

_NC_CACHE = {}


def _ap3(dram_ap, off, ap):
    return bass.AP(tensor=dram_ap.tensor, offset=dram_ap.offset + off, ap=ap)


def _sb(tile_ap, off, ap):
    """Manual sub-AP of an SBUF tile."""
    return bass.AP(tensor=tile_ap.tensor, offset=tile_ap.offset + off, ap=ap)


def build_nc():
    if "nc" in _NC_CACHE:
        return _NC_CACHE["nc"]
    nc = bacc.Bacc("TRN2", target_bir_lowering=False, debug=False,
                   num_devices=NCORES)
    ins = {"woT": nc.dram_tensor("woT", [GO, D], BF16,
                                 kind="ExternalInput").ap(),
           "C": nc.dram_tensor("C", [P, L], BF16, kind="ExternalInput").ap(),
           "S": nc.dram_tensor("S", [P, L], BF16, kind="ExternalInput").ap()}
    for nm, shp in (("xT", [D, L]), ("wqT", [D, GO]), ("wkT", [D, KVO]),
                    ("wvT", [D, KVO])):
        for c in "hl":
            ins[nm + c] = nc.dram_tensor(nm + c, shp, F8,
                                         kind="ExternalInput").ap()
    y = nc.dram_tensor("y", [L, D], BF16, kind="ExternalOutput").ap()
    with tile.TileContext(nc) as tc:
        _build_kernel(nc, tc, ins, y)
    nc.compile()
    _NC_CACHE["nc"] = nc
    return nc


def _rope(nc, pool, raw, C, S, out):
    """out = raw*C + shuffle16(raw)*S   (all [128, L] bf16)."""
    shuf = pool.tile([P, L], BF16, tag="rope_shuf")
    mask = [(p ^ 16) for p in range(32)]
    nc.vector.stream_shuffle(shuf[:], raw[:], mask)
    m1 = pool.tile([P, L], BF16, tag="rope_m1")
    nc.vector.tensor_mul(m1[:], raw[:], C[:])
    m2 = pool.tile([P, L], BF16, tag="rope_m2")
    nc.vector.tensor_mul(m2[:], shuf[:], S[:])
    nc.vector.tensor_add(out[:], m1[:], m2[:])


def _build_kernel(nc, tc, ins, y):
    import contextlib
    ctx = contextlib.ExitStack()
    with ctx:
        const = ctx.enter_context(tc.tile_pool(name="const", bufs=1))
        qt_pool = ctx.enter_context(tc.tile_pool(name="qt", bufs=1))
        kt_pool = ctx.enter_context(tc.tile_pool(name="kt", bufs=1))
        v_pool = ctx.enter_context(tc.tile_pool(name="vx", bufs=1))
        at_pool = ctx.enter_context(tc.tile_pool(name="at", bufs=1))
        wo_pool = ctx.enter_context(tc.tile_pool(name="wo", bufs=1))
        QT = [qt_pool.tile([P, L], BF16, tag=f"qt{t}", name=f"qt{t}")
              for t in range(8)]
        KT = [kt_pool.tile([P, L], BF16, tag=f"kt{i}", name=f"kt{i}")
              for i in range(2)]
        # Vst[kb]: [1|V0|1|V1|1|V2|1|V3|1], 9 64-col groups
        Vst = [v_pool.tile([P, 9, 64], BF16, tag=f"v{b_}", name=f"v{b_}")
               for b_ in range(LB)]
        AT = [at_pool.tile([P, L], BF16, tag=f"at{t}", name=f"at{t}")
              for t in range(8)]
        WO = [wo_pool.tile([P, 4 * D], BF16, tag=f"wo{j}", name=f"wo{j}")
              for j in range(2)]
        Ctab = const.tile([P, L], BF16, tag="Ctab", name="Ctab")
        Stab = const.tile([P, L], BF16, tag="Stab", name="Stab")
        dum_src = const.tile([P, P], BF16, tag="dum_src", name="dum_src")
        nc.vector.memset(dum_src[:], 0.0)
        for b_ in range(LB):
            for j in range(5):
                nc.vector.memset(Vst[b_][:, 2 * j, :], SX * SW)

        with tc.tile_pool(name="xt", bufs=1) as xt_pool, \
             tc.tile_pool(name="wst", bufs=1) as wst, \
             tc.tile_pool(name="ev", bufs=2) as ev:
            X8 = {c: xt_pool.tile([P, NI, L], F8, tag=f"x{c}", name=f"x{c}")
                  for c in "hl"}
            WV8 = {c: wst.tile([P, NI, KVO], F8, tag=f"wv{c}", name=f"wv{c}")
                   for c in "hl"}
            WK8 = {c: wst.tile([P, NI, KVO], F8, tag=f"wk{c}", name=f"wk{c}")
                   for c in "hl"}
            WQ8 = [{c: wst.tile([P, NI, 512], F8, tag=f"wq{og}{c}",
                               name=f"wq{og}{c}") for c in "hl"}
                   for og in range(2)]

            def ld_x(c, c0, cn):
                nc.sync.dma_start(
                    X8[c][:, c0:c0 + cn, :],
                    _ap3(ins["xT" + c], c0 * P * L,
                         [[L, P], [P * L, cn], [1, L]]))

            def ld_wq(og, c, c0, cn):
                nc.sync.dma_start(
                    WQ8[og][c][:, c0:c0 + cn, :],
                    _ap3(ins["wqT" + c], og * 512 + c0 * P * GO,
                         [[GO, P], [P * GO, cn], [1, 512]]))

            def ld_wkv(name, dst, c, c0, cn):
                nc.sync.dma_start(
                    dst[c][:, c0:c0 + cn, :],
                    _ap3(ins[name + c], c0 * P * KVO,
                         [[KVO, P], [P * KVO, cn], [1, KVO]]))

            # strict consumption order: Q0 pass0 (wq0h+xh), pass1 (wq0l),
            # pass2 (xl), rope tables, K, V, og1 weights, wo
            ld_wq(0, "h", 0, 2); ld_x("h", 0, 2); ld_wq(0, "h", 2, 2)
            ld_x("h", 2, 2); ld_wq(0, "h", 4, 4); ld_x("h", 4, 4)
            ld_wq(0, "h", 8, 8); ld_x("h", 8, 4); ld_wq(0, "l", 0, 8)
            ld_x("h", 12, 4); ld_wq(0, "l", 8, 8)
            ld_x("l", 0, 8); ld_x("l", 8, 8)
            nc.sync.dma_start(Ctab[:], ins["C"][:, :])
            nc.sync.dma_start(Stab[:], ins["S"][:, :])
            ld_wkv("wkT", WK8, "h", 0, 16); ld_wkv("wkT", WK8, "l", 0, 16)
            ld_wkv("wvT", WV8, "h", 0, 16); ld_wkv("wvT", WV8, "l", 0, 16)
            ld_wq(1, "h", 0, 8); ld_wq(1, "h", 8, 8)
            ld_wq(1, "l", 0, 8); ld_wq(1, "l", 8, 8)
            for j in range(2):
                for h in range(2):
                    nc.sync.dma_start(
                        WO[j][:, h * 2 * D:(h + 1) * 2 * D],
                        _ap3(ins["woT"], (j * 4 + h * 2) * P * D,
                             [[D, P], [P * D, 2], [1, D]]))

            # ---------- phase 1: Q og0, K, V projections -----------------
            with tc.tile_pool(name="ps1", bufs=1, space="PSUM") as ps1:
                dumps = ps1.tile([P, P], F32, tag="ps1_7", name="dumps")

                def dummies(n):
                    # keep the PE continuously busy across a known stall so
                    # the p-state never drops (each is [128,128], ~53 ns)
                    for _ in range(n):
                        nc.tensor.matmul(dumps[:], dum_src[:], dum_src[:],
                                         start=True, stop=True)

                dummies(45)
                # 3-term fp8 split: xh@wh + xh@wl + xl@wh, DoubleRow pairs
                # of 128-chunks (8 dr-chunks of 256 contraction each)
                PASSES = (("h", "h"), ("h", "l"), ("l", "h"))
                psq = [ps1.tile([P, 448], F32, tag=f"ps1_{j}",
                                name=f"psq0_{j}") for j in range(8)]

                def q_mm(og, psum, p, i, ob, h2):
                    xc, wc = PASSES[p]
                    nc.tensor.matmul(
                        psum[ob * 2 + h2][:],
                        WQ8[og][wc][:, 2 * i:2 * i + 2, ob * P:(ob + 1) * P],
                        X8[xc][:, 2 * i:2 * i + 2,
                               h2 * 448:(h2 + 1) * 448],
                        start=(p == 0 and i == 0),
                        stop=(p == 2 and i == NI // 2 - 1),
                        perf_mode=mybir.MatmulPerfMode.DoubleRow)

                for p in range(3):
                    for i in range(NI // 2):
                        for ob in range(4):
                            for h2 in range(2):
                                q_mm(0, psq, p, i, ob, h2)
                for ob in range(4):
                    raw = ev.tile([P, L], BF16, tag="qraw")
                    for h2 in range(2):
                        nc.vector.tensor_copy(raw[:, h2 * 448:(h2 + 1) * 448],
                                              psq[ob * 2 + h2][:])
                    _rope(nc, ev, raw, Ctab, Stab, QT[ob])

                psk = [ps1.tile([P, 448], F32, tag=f"ps1_{j}", name=f"psk{j}")
                       for j in range(4)]
                for p in range(3):
                    xc, wc = PASSES[p]
                    for i in range(NI // 2):
                        for ob in range(2):
                            for h2 in range(2):
                                nc.tensor.matmul(
                                    psk[ob * 2 + h2][:],
                                    WK8[wc][:, 2 * i:2 * i + 2,
                                            ob * P:(ob + 1) * P],
                                    X8[xc][:, 2 * i:2 * i + 2,
                                           h2 * 448:(h2 + 1) * 448],
                                    start=(p == 0 and i == 0),
                                    stop=(p == 2 and i == NI // 2 - 1),
                                    perf_mode=mybir.MatmulPerfMode.DoubleRow)
                for ob in range(2):
                    raw = ev.tile([P, L], BF16, tag="kraw")
                    for h2 in range(2):
                        nc.vector.tensor_copy(raw[:, h2 * 448:(h2 + 1) * 448],
                                              psk[ob * 2 + h2][:])
                    _rope(nc, ev, raw, Ctab, Stab, KT[ob])

                psv = [ps1.tile([P, KVO], F32, tag=f"ps1_{b_}",
                                name=f"psv{b_}") for b_ in range(LB)]
                for p in range(3):
                    xc, wc = PASSES[p]
                    for i in range(NI // 2):
                        for b_ in range(LB):
                            nc.tensor.matmul(
                                psv[b_][:],
                                X8[xc][:, 2 * i:2 * i + 2,
                                       b_ * P:(b_ + 1) * P],
                                WV8[wc][:, 2 * i:2 * i + 2, :],
                                start=(p == 0 and i == 0),
                                stop=(p == 2 and i == NI // 2 - 1),
                                perf_mode=mybir.MatmulPerfMode.DoubleRow)
                            if p == 2 and i == NI // 2 - 1:
                                nc.scalar.copy(Vst[b_][:, 1:9:2, :],
                                               psv[b_][:])
                dumps2 = ps1.tile([P, P], F32, tag="ps1_7", name="dumps2")
                for _ in range(50):
                    nc.tensor.matmul(dumps2[:], dum_src[:], dum_src[:],
                                     start=True, stop=True)

            # ---------- phase 2: attention with og1 Q-proj filler --------
            with tc.tile_pool(name="uatt", bufs=6) as upool, \
                 tc.tile_pool(name="rec", bufs=2) as recpool, \
                 tc.tile_pool(name="pss", bufs=3, space="PSUM") as pss, \
                 tc.tile_pool(name="psav", bufs=2, space="PSUM") as pspool, \
                 tc.tile_pool(name="ps2", bufs=1, space="PSUM") as ps2:

                # og1 Q projection as a drainable list of PE work items
                og1_state = {"ps": None, "raw": None}

                def og1_mm(ob, h2, p, i):
                    if p == 0 and i == 0:
                        og1_state["ps"] = ps2.tile([P, 448], F32, tag="q1",
                                                   name=f"psq1_{ob}_{h2}")
                    xc, wc = PASSES[p]
                    nc.tensor.matmul(
                        og1_state["ps"][:],
                        WQ8[1][wc][:, 2 * i:2 * i + 2, ob * P:(ob + 1) * P],
                        X8[xc][:, 2 * i:2 * i + 2, h2 * 448:(h2 + 1) * 448],
                        start=(p == 0 and i == 0),
                        stop=(p == 2 and i == NI // 2 - 1),
                        perf_mode=mybir.MatmulPerfMode.DoubleRow)

                def og1_evac(ob, h2):
                    if h2 == 0:
                        og1_state["raw"] = ev.tile([P, L], BF16, tag="qraw", name="qraw1")
                    nc.vector.tensor_copy(
                        og1_state["raw"][:, h2 * 448:(h2 + 1) * 448],
                        og1_state["ps"][:])
                    if h2 == 1:
                        _rope(nc, ev, og1_state["raw"], Ctab, Stab, QT[4 + ob])

                og1_work = []
                for ob in range(4):
                    for h2 in range(2):
                        for p in range(3):
                            for i in range(NI // 2):
                                og1_work.append(
                                    lambda ob=ob, h2=h2, p=p, i=i:
                                    og1_mm(ob, h2, p, i))
                        og1_work.append(
                            lambda ob=ob, h2=h2: og1_evac(ob, h2))

                def att_dummies(n):
                    dq = ps2.tile([P, 448], F32, tag="q1", name="dq")
                    for _ in range(n):
                        nc.tensor.matmul(dq[:, 0:P], dum_src[:], dum_src[:],
                                         start=True, stop=True)

                def filler(pair):
                    if og1_work:
                        for _ in range(5 if pair % 2 == 0 else 4):
                            if og1_work:
                                og1_work.pop(0)()
                    else:
                        att_dummies(16)

                psav = {}

                def scores(t, s, bi):
                    no = s * 64
                    blocks = BLOCKS[bi * EXPB:(bi + 1) * EXPB]
                    ng = len(blocks)
                    ps_s = pss.tile([P, EXPB, P], F32, tag="s",
                                    name=f"s{t}_{s}_{bi}")
                    for j, (qb, kb) in enumerate(blocks):
                        nc.tensor.matmul(
                            ps_s[:, j, :],
                            KT[t // 4][no:no + 64, kb * P:(kb + 1) * P],
                            QT[t][no:no + 64, qb * P:(qb + 1) * P],
                            start=True, stop=True,
                            tile_position=(no, 0))
                    U = upool.tile([P, EXPB, P], BF16, tag="u",
                                   name=f"u{t}_{s}_{bi}")
                    nc.scalar.activation(U[:, 0:ng, :], ps_s[:, 0:ng, :],
                                         AF.Exp, scale=0.125)
                    for j, (qb, kb) in enumerate(blocks):
                        if qb == kb:
                            nc.gpsimd.affine_select(
                                out=U[:, j, :], in_=U[:, j, :],
                                compare_op=ALU.is_ge, fill=0.0,
                                base=0, channel_multiplier=-1,
                                pattern=[[1, P]])
                    return U, blocks

                def avs(t, s, bi, U, blocks):
                    kv = 2 * (t // 4) + s
                    if (t, s) not in psav:
                        psav[(t, s)] = pspool.tile([P, L], F32, tag="av",
                                                   name=f"av{t}_{s}")
                    for j, (qb, kb) in enumerate(blocks):
                        nc.tensor.matmul(
                            psav[(t, s)][:, qb * P:(qb + 1) * P],
                            _sb(Vst[kb], kv * P + (1 - s) * 64,
                                [[576, P], [1, P]]),
                            U[:, j, :],
                            start=(kb == 0), stop=(kb == qb))
                    if bi == NBATCH - 1:
                        epilogue(t, s)
                        if not og1_work:
                            att_dummies(14)

                def epilogue(t, s):
                    no, so = s * 64, (1 - s) * 64
                    rec = recpool.tile([P, L], F32, tag="rec",
                                       name=f"rec{t}_{s}")
                    nc.vector.reciprocal(rec[so:so + 64, :],
                                         psav[(t, s)][so:so + 64, :])
                    rec2 = recpool.tile([P, L], F32, tag="rec2",
                                        name=f"rec2{t}_{s}")
                    nc.sync.dma_start(rec2[no:no + 64, :],
                                      rec[so:so + 64, :])
                    nc.vector.tensor_mul(AT[t][no:no + 64, :],
                                         psav[(t, s)][no:no + 64, :],
                                         rec2[no:no + 64, :])

                # flat software pipeline: AVs lag scores by 2 batch-pairs so
                # they never reach the PE queue before their exp is done
                fifo = []
                pair = 0
                for t in range(8):
                    for bi in range(NBATCH):
                        for s in range(2):
                            fifo.append((t, s, bi) + scores(t, s, bi))
                        filler(pair)
                        pair += 1
                        while len(fifo) > 4:
                            avs(*fifo.pop(0))
                while og1_work:
                    og1_work.pop(0)()
                while fifo:
                    avs(*fifo.pop(0))
                    att_dummies(12)
                att_dummies(100)

        # ---------------- phase 3: out projection ------------------------
        with tc.tile_pool(name="osb", bufs=2) as osb, \
             tc.tile_pool(name="pso", bufs=1, space="PSUM") as pso:
            def p3_mm(ps, oc, b_, ic):
                nc.tensor.matmul(
                    ps[:], AT[ic][:, b_ * P:(b_ + 1) * P],
                    WO[ic // 4][:, (ic % 4) * D + oc * 512:
                                (ic % 4) * D + (oc + 1) * 512],
                    start=(ic == 0), stop=(ic == 7))

            for oc in range(4):
                ob_t = osb.tile([P, LB, 512], BF16, tag="ot", name=f"ot{oc}")
                # 4 psum banks (the ones freed before the last epilogues);
                # for oc 0 defer ic=7 (needs the last head's AT) past a full
                # wave of ic 0..6 so the PE never waits on the epilogue tail
                for w0, wn in ((0, 4), (4, 3)):
                    ps_w = []
                    for b_ in range(w0, w0 + wn):
                        ps = pso.tile([P, 512], F32, tag=f"pso{b_ % 4}",
                                      name=f"pso{oc}_{b_}")
                        ps_w.append(ps)
                        n_ic = 7 if oc == 0 else 8
                        for ic in range(n_ic):
                            p3_mm(ps, oc, b_, ic)
                        if oc != 0:
                            nc.scalar.copy(ob_t[:, b_, :], ps[:])
                    if oc == 0:
                        for j, b_ in enumerate(range(w0, w0 + wn)):
                            p3_mm(ps_w[j], oc, b_, 7)
                            nc.scalar.copy(ob_t[:, b_, :], ps_w[j][:])
                nc.scalar.dma_start(
                    _ap3(y, oc * 512, [[D, P], [P * D, LB], [1, 512]]),
                    ob_t[:])


# ---------------------------------------------------------------- host side
ROPE_BASE = 10000.0
_ROPE_PERM = np.concatenate([
    np.arange(0, 32, 2), np.arange(1, 32, 2),
    np.arange(32, 64, 2), np.arange(33, 64, 2)])
# local q head at (tile t, slot s) = _LOCAL_HEADS[2*t + s]
_LOCAL_HEADS = [8 * (t // 4) + 4 * s + t % 4 for t in range(8) for s in range(2)]


F8NP = (ml_dtypes.float8_e4m3fn if hasattr(ml_dtypes, "float8_e4m3fn")
        else ml_dtypes.float8_e4m3)


def _split8(a, scale):
    """hi/lo fp8 split at a single scale: a ~= (hi + lo)/scale."""
    hi = np.asarray(a * scale, dtype=F8NP)
    lo = np.asarray(a * scale - hi.astype(np.float32), dtype=F8NP)
    return np.ascontiguousarray(hi), np.ascontiguousarray(lo)


def _cos_sin_tables(temporal_pos, structural_pos):
    inv = (1.0 / ROPE_BASE) ** (np.arange(16, dtype=np.float64) / 16.0)
    tabs = {}
    for name, pos in (("t", temporal_pos), ("s", structural_pos)):
        ang = np.outer(inv, np.asarray(pos, dtype=np.float64))  # [16, L]
        tabs[name] = (np.cos(ang), np.sin(ang))
    ct, st = tabs["t"]
    cs, ss = tabs["s"]
    # 1/(SX*SW) folds the fp8 psum scale out of the roped q/k
    C64 = np.concatenate([ct, ct, cs, cs], axis=0) / (SX * SW)
    S64 = np.concatenate([-st, st, -ss, ss], axis=0) / (SX * SW)
    C = np.tile(C64, (2, 1)).astype(ml_dtypes.bfloat16)
    S = np.tile(S64, (2, 1)).astype(ml_dtypes.bfloat16)
    return np.ascontiguousarray(C), np.ascontiguousarray(S)


def make_in_maps(x, wq, wk, wv, wo, temporal_pos, structural_pos):
    bf = ml_dtypes.bfloat16
    x = np.asarray(x, dtype=np.float32)
    wq4 = np.asarray(wq, dtype=np.float32).reshape(HQ, HD, D)
    wk4 = np.asarray(wk, dtype=np.float32).reshape(HKV, HD, D)
    wv4 = np.asarray(wv, dtype=np.float32).reshape(HKV, HD, D)
    woT = np.asarray(wo, dtype=np.float32).T  # [D(in head dims), D(out)]
    C, S = _cos_sin_tables(temporal_pos, structural_pos)

    in_maps = []
    for c in range(NCORES):
        b, g = divmod(c, 2)
        heads = [16 * g + h for h in _LOCAL_HEADS]
        wq_g = wq4[heads][:, _ROPE_PERM, :].reshape(GO, D)
        wk_g = wk4[4 * g:4 * g + 4][:, _ROPE_PERM, :].reshape(KVO, D)
        wv_g = wv4[4 * g:4 * g + 4].reshape(KVO, D)
        woT_g = np.concatenate([woT[64 * h:64 * h + 64, :] for h in heads])
        m = {"woT": np.ascontiguousarray(woT_g).astype(bf), "C": C, "S": S}
        m["xTh"], m["xTl"] = _split8(np.ascontiguousarray(x[b].T), SX)
        m["wqTh"], m["wqTl"] = _split8(np.ascontiguousarray(wq_g.T), SW)
        m["wkTh"], m["wkTl"] = _split8(np.ascontiguousarray(wk_g.T), SW)
        m["wvTh"], m["wvTl"] = _split8(np.ascontiguousarray(wv_g.T), SW)
        in_maps.append(m)
    return in_maps


def kernel(x, wq, wk, wv, wo, temporal_pos, structural_pos, _trace=False):
    nc = build_nc()
    in_maps = make_in_maps(x, wq, wk, wv, wo, temporal_pos, structural_pos)
    res = bass_utils.run_bass_kernel_spmd(
        nc, in_maps, core_ids=list(range(NCORES)), trace=_trace)
    out = np.stack([
        np.asarray(res.results[2 * b]["y"], dtype=np.float32)
        + np.asarray(res.results[2 * b + 1]["y"], dtype=np.float32)
        for b in range(B)])
    kernel.last_result = res
    return out


# revision 45
# speedup vs baseline: 2.1224x; 1.0758x over previous
"""GQA attention with 2D RoPE on 8 TRN2 NeuronCores.

Sharding: batch data-parallel x4  X  head-group tensor-parallel x2.
Core c handles batch b=c//2 and head group g=c%2 (16 Q heads, 4 KV heads).
Each core writes its PARTIAL out-projection y_g = A_g @ woT_g  [896, 2048]
(bf16); the host sums the two partials per batch (no device collective).

All weights/activations enter as bf16 (host-cast).  RoPE cos/sin tables are
host-computed.  Q heads are host-permuted so the 4 Q heads of each KV head
live in the same partition half as that KV head (no K duplication): QT tile
t, slot s  <->  local q head  8*(t//4) + 4*s + t%4, kv head 2*(t//4)+s.

The Q/K/V projections run as 3-term fp8 splits (x = xh+xl, w = wh+wl at a
shared scale, dropping the xl*wl term) with DoubleRow perf mode, which this
PE executes at 4x bf16 throughput; accuracy beats plain bf16.  The 1/512
psum scale folds into the host RoPE tables (q, k) and the Vst ones columns
(denominator), so no extra scaling ops exist on device.

Attention uses 128-wide q blocks, 28 causal (qb, kb) blocks per head in
qb-major order, batched 8-at-a-time so exp runs as 4 big Activation
instructions per head (the Activation engine is the attention bottleneck).
The two slots of each QT tile run as two interleaved streams in a flat
software pipeline where AVs lag scores by 2 batch-pairs; each tile's tail
is flushed early so softmax epilogues overlap the next tile's scores.

V staging per key block kb: Vst[kb] [128 keys, 9, 64] = [1|V0|1|V1|...|V3|1]
(64-col groups).  AV lhsT for kv head k, slot s = cols [128k + 64*(1-s),
+128) which is [V_k|1] for s=0 (numerators rows 0:64, denominator sums rows
64:128) and [1|V_k] for s=1.
"""

import numpy as np
import ml_dtypes

import concourse.bass as bass
import concourse.tile as tile
import concourse.mybir as mybir
from concourse import bacc
from concourse import bass_utils

F32 = mybir.dt.float32
BF16 = mybir.dt.bfloat16
F8 = mybir.dt.float8e4
AF = mybir.ActivationFunctionType
ALU = mybir.AluOpType

B, L, D = 4, 896, 2048
HQ, HKV, HD = 32, 8, 64
NCORES = 8
GO = D // 2          # 1024 q-out dims per core
KVO = HKV * HD // 2  # 256 kv-out dims per core
P = 128
NI = D // P          # 16 contraction chunks
LB = L // P          # 7 key/q blocks

# causal blocks per head, qb-major: [(qb, kb)], kb <= qb
BLOCKS = [(qb, kb) for qb in range(LB) for kb in range(qb + 1)]
SX, SW = 8.0, 64.0   # fp8 operand scales (psum carries SX*SW = 512x)
EXPB = 8             # blocks per exp batch
NBATCH = (len(BLOCKS) + EXPB - 1) // EXPB   # 4 batches: 8,8,8,4

_NC_CACHE = {}


def _ap3(dram_ap, off, ap):
    return bass.AP(tensor=dram_ap.tensor, offset=dram_ap.offset + off, ap=ap)


def _sb(tile_ap, off, ap):
    """Manual sub-AP of an SBUF tile."""
    return bass.AP(tensor=tile_ap.tensor, offset=tile_ap.offset + off, ap=ap)


def build_nc():
    if "nc" in _NC_CACHE:
        return _NC_CACHE["nc"]
    nc = bacc.Bacc("TRN2", target_bir_lowering=False, debug=False,
                   num_devices=NCORES)
    ins = {"woT": nc.dram_tensor("woT", [GO, D], BF16,
                                 kind="ExternalInput").ap(),
           "C": nc.dram_tensor("C", [P, L], BF16, kind="ExternalInput").ap(),
           "S": nc.dram_tensor("S", [P, L], BF16, kind="ExternalInput").ap(),
           "SW": nc.dram_tensor("SW", [P, P], BF16,
                                kind="ExternalInput").ap()}
    for nm, shp in (("xT", [D, L]), ("wqT", [D, GO]), ("wkT", [D, KVO]),
                    ("wvT", [D, KVO])):
        for c in "hl":
            ins[nm + c] = nc.dram_tensor(nm + c, shp, F8,
                                         kind="ExternalInput").ap()
    y = nc.dram_tensor("y", [L, D], BF16, kind="ExternalOutput").ap()
    with tile.TileContext(nc) as tc:
        _build_kernel(nc, tc, ins, y)
    nc.compile()
    _NC_CACHE["nc"] = nc
    return nc


def _rope(nc, pool, raw, C, S, out):
    """out = raw*C + shuffle16(raw)*S   (all [128, L] bf16)."""
    shuf = pool.tile([P, L], BF16, tag="rope_shuf")
    mask = [(p ^ 16) for p in range(32)]
    nc.vector.stream_shuffle(shuf[:], raw[:], mask)
    m1 = pool.tile([P, L], BF16, tag="rope_m1")
    nc.vector.tensor_mul(m1[:], raw[:], C[:])
    m2 = pool.tile([P, L], BF16, tag="rope_m2")
    nc.vector.tensor_mul(m2[:], shuf[:], S[:])
    nc.vector.tensor_add(out[:], m1[:], m2[:])


def _build_kernel(nc, tc, ins, y):
    import contextlib
    ctx = contextlib.ExitStack()
    with ctx:
        const = ctx.enter_context(tc.tile_pool(name="const", bufs=1))
        qt_pool = ctx.enter_context(tc.tile_pool(name="qt", bufs=1))
        kt_pool = ctx.enter_context(tc.tile_pool(name="kt", bufs=1))
        v_pool = ctx.enter_context(tc.tile_pool(name="vx", bufs=1))
        at_pool = ctx.enter_context(tc.tile_pool(name="at", bufs=1))
        wo_pool = ctx.enter_context(tc.tile_pool(name="wo", bufs=1))
        QT = [qt_pool.tile([P, L], BF16, tag=f"qt{t}", name=f"qt{t}")
              for t in range(8)]
        KT = [kt_pool.tile([P, L], BF16, tag=f"kt{i}", name=f"kt{i}")
              for i in range(2)]
        # Vst[kb]: [1|V0|1|V1|1|V2|1|V3|1], 9 64-col groups
        Vst = [v_pool.tile([P, 9, 64], BF16, tag=f"v{b_}", name=f"v{b_}")
               for b_ in range(LB)]
        AT = [at_pool.tile([P, L], BF16, tag=f"at{t}", name=f"at{t}")
              for t in range(8)]
        WO = [wo_pool.tile([P, 4 * D], BF16, tag=f"wo{j}", name=f"wo{j}")
              for j in range(2)]
        Ctab = const.tile([P, L], BF16, tag="Ctab", name="Ctab")
        Stab = const.tile([P, L], BF16, tag="Stab", name="Stab")
        dum_src = const.tile([P, P], BF16, tag="dum_src", name="dum_src")
        nc.vector.memset(dum_src[:], 0.0)
        SWt = const.tile([P, P], BF16, tag="SWt", name="SWt")
        for b_ in range(LB):
            for j in range(5):
                nc.vector.memset(Vst[b_][:, 2 * j, :], SX * SW)

        with tc.tile_pool(name="xt", bufs=1) as xt_pool, \
             tc.tile_pool(name="wst", bufs=1) as wst, \
             tc.tile_pool(name="ev", bufs=2) as ev:
            X8 = {c: xt_pool.tile([P, NI, L], F8, tag=f"x{c}", name=f"x{c}")
                  for c in "hl"}
            WV8 = {c: wst.tile([P, NI, KVO], F8, tag=f"wv{c}", name=f"wv{c}")
                   for c in "hl"}
            WK8 = {c: wst.tile([P, NI, KVO], F8, tag=f"wk{c}", name=f"wk{c}")
                   for c in "hl"}
            WQ8 = [{c: wst.tile([P, NI, 512], F8, tag=f"wq{og}{c}",
                               name=f"wq{og}{c}") for c in "hl"}
                   for og in range(2)]

            def ld_x(c, c0, cn):
                nc.sync.dma_start(
                    X8[c][:, c0:c0 + cn, :],
                    _ap3(ins["xT" + c], c0 * P * L,
                         [[L, P], [P * L, cn], [1, L]]))

            def ld_wq(og, c, c0, cn):
                nc.sync.dma_start(
                    WQ8[og][c][:, c0:c0 + cn, :],
                    _ap3(ins["wqT" + c], og * 512 + c0 * P * GO,
                         [[GO, P], [P * GO, cn], [1, 512]]))

            def ld_wkv(name, dst, c, c0, cn):
                nc.sync.dma_start(
                    dst[c][:, c0:c0 + cn, :],
                    _ap3(ins[name + c], c0 * P * KVO,
                         [[KVO, P], [P * KVO, cn], [1, KVO]]))

            # strict consumption order: Q0 pass0 (wq0h+xh), pass1 (wq0l),
            # pass2 (xl), og1 weights, rope tables, K, V, wo
            for c0 in range(0, 16, 4):
                ld_wq(0, "h", c0, 4); ld_x("h", c0, 4); ld_wq(0, "l", c0, 4)
            for c0 in range(0, 16, 4):
                ld_x("l", c0, 4)
            ld_wq(1, "h", 0, 4); ld_wq(1, "h", 4, 4)
            ld_wq(1, "h", 8, 4); ld_wq(1, "h", 12, 4)
            ld_wq(1, "l", 0, 8); ld_wq(1, "l", 8, 8)
            nc.sync.dma_start(Ctab[:], ins["C"][:, :])
            nc.sync.dma_start(Stab[:], ins["S"][:, :])
            nc.sync.dma_start(SWt[:], ins["SW"][:, :])
            ld_wkv("wkT", WK8, "h", 0, 16); ld_wkv("wkT", WK8, "l", 0, 16)
            ld_wkv("wvT", WV8, "h", 0, 16); ld_wkv("wvT", WV8, "l", 0, 16)
            for j in range(2):
                for h in range(2):
                    nc.sync.dma_start(
                        WO[j][:, h * 2 * D:(h + 1) * 2 * D],
                        _ap3(ins["woT"], (j * 4 + h * 2) * P * D,
                             [[D, P], [P * D, 2], [1, D]]))

            # ---------- phase 1: Q og0, K, V projections -----------------
            with tc.tile_pool(name="ps1", bufs=1, space="PSUM") as ps1:
                dumps = ps1.tile([P, P], F32, tag="ps1_7", name="dumps")

                def dummies(n):
                    # keep the PE continuously busy across a known stall so
                    # the p-state never drops (each is [128,128], ~53 ns)
                    for _ in range(n):
                        nc.tensor.matmul(dumps[:], dum_src[:], dum_src[:],
                                         start=True, stop=True)

                dummies(38)
                # 3-term fp8 split: xh@wh + xh@wl + xl@wh, DoubleRow pairs
                # of 128-chunks (8 dr-chunks of 256 contraction each)
                PASSES = (("h", "h"), ("h", "l"), ("l", "h"))
                psq = [ps1.tile([P, 448], F32, tag=f"ps1_{j}",
                                name=f"psq0_{j}") for j in range(8)]

                def q_mm(og, psum, p, i, ob, h2):
                    xc, wc = PASSES[p]
                    nc.tensor.matmul(
                        psum[ob * 2 + h2][:],
                        WQ8[og][wc][:, 2 * i:2 * i + 2, ob * P:(ob + 1) * P],
                        X8[xc][:, 2 * i:2 * i + 2,
                               h2 * 448:(h2 + 1) * 448],
                        start=(p == 0 and i == 0),
                        stop=(p == 2 and i == NI // 2 - 1),
                        perf_mode=mybir.MatmulPerfMode.DoubleRow)

                # interleave passes 0/1 per chunk: pass0 alone is gated by
                # the wq0h+xh DMA stream; pass1 reuses xh so it fills the gaps
                for i in range(NI // 2):
                    for p in range(2):
                        for ob in range(4):
                            for h2 in range(2):
                                q_mm(0, psq, p, i, ob, h2)
                for i in range(NI // 2):
                    for ob in range(4):
                        for h2 in range(2):
                            q_mm(0, psq, 2, i, ob, h2)
                for ob in range(4):
                    raw = ev.tile([P, L], BF16, tag="qraw")
                    for h2 in range(2):
                        nc.vector.tensor_copy(raw[:, h2 * 448:(h2 + 1) * 448],
                                              psq[ob * 2 + h2][:])
                    _rope(nc, ev, raw, Ctab, Stab, QT[ob])

                # og1 Q projection (fp8 DR), tiles 4..7
                psq1 = [ps1.tile([P, 448], F32, tag=f"ps1_{j}",
                                 name=f"psq1_{j}") for j in range(8)]
                for p in range(3):
                    for i in range(NI // 2):
                        for ob in range(4):
                            for h2 in range(2):
                                q_mm(1, psq1, p, i, ob, h2)
                for ob in range(4):
                    raw = ev.tile([P, L], BF16, tag="qraw")
                    for h2 in range(2):
                        nc.vector.tensor_copy(raw[:, h2 * 448:(h2 + 1) * 448],
                                              psq1[ob * 2 + h2][:])
                    _rope(nc, ev, raw, Ctab, Stab, QT[4 + ob])

                psk = [ps1.tile([P, 448], F32, tag=f"ps1_{j}", name=f"psk{j}")
                       for j in range(4)]
                for p in range(3):
                    xc, wc = PASSES[p]
                    for i in range(NI // 2):
                        for ob in range(2):
                            for h2 in range(2):
                                nc.tensor.matmul(
                                    psk[ob * 2 + h2][:],
                                    WK8[wc][:, 2 * i:2 * i + 2,
                                            ob * P:(ob + 1) * P],
                                    X8[xc][:, 2 * i:2 * i + 2,
                                           h2 * 448:(h2 + 1) * 448],
                                    start=(p == 0 and i == 0),
                                    stop=(p == 2 and i == NI // 2 - 1),
                                    perf_mode=mybir.MatmulPerfMode.DoubleRow)
                for ob in range(2):
                    raw = ev.tile([P, L], BF16, tag="kraw")
                    for h2 in range(2):
                        nc.vector.tensor_copy(raw[:, h2 * 448:(h2 + 1) * 448],
                                              psk[ob * 2 + h2][:])
                    _rope(nc, ev, raw, Ctab, Stab, KT[ob])

                psv = [ps1.tile([P, KVO], F32, tag=f"ps1_{b_}",
                                name=f"psv{b_}") for b_ in range(LB)]
                for p in range(3):
                    xc, wc = PASSES[p]
                    for i in range(NI // 2):
                        for b_ in range(LB):
                            nc.tensor.matmul(
                                psv[b_][:],
                                X8[xc][:, 2 * i:2 * i + 2,
                                       b_ * P:(b_ + 1) * P],
                                WV8[wc][:, 2 * i:2 * i + 2, :],
                                start=(p == 0 and i == 0),
                                stop=(p == 2 and i == NI // 2 - 1),
                                perf_mode=mybir.MatmulPerfMode.DoubleRow)
                            if p == 2 and i == NI // 2 - 1:
                                nc.scalar.copy(Vst[b_][:, 1:9:2, :],
                                               psv[b_][:])


            # ---------- phase 2: attention ------------------------------
            with tc.tile_pool(name="uatt", bufs=6) as upool, \
                 tc.tile_pool(name="rec", bufs=2) as recpool, \
                 tc.tile_pool(name="pss", bufs=2, space="PSUM") as pss, \
                 tc.tile_pool(name="psav", bufs=2, space="PSUM") as pspool:

                psav = {}

                def scores(t, s, bi):
                    no = s * 64
                    blocks = BLOCKS[bi * EXPB:(bi + 1) * EXPB]
                    ng = len(blocks)
                    ps_s = pss.tile([P, EXPB, P], F32, tag="s",
                                    name=f"s{t}_{s}_{bi}")
                    for j, (qb, kb) in enumerate(blocks):
                        nc.tensor.matmul(
                            ps_s[:, j, :],
                            KT[t // 4][no:no + 64, kb * P:(kb + 1) * P],
                            QT[t][no:no + 64, qb * P:(qb + 1) * P],
                            start=True, stop=True,
                            tile_position=(no, 0))
                    U = upool.tile([P, EXPB, P], BF16, tag="u",
                                   name=f"u{t}_{s}_{bi}")
                    nc.scalar.activation(U[:, 0:ng, :], ps_s[:, 0:ng, :],
                                         AF.Exp, scale=0.125)
                    for j, (qb, kb) in enumerate(blocks):
                        if qb == kb:
                            nc.gpsimd.affine_select(
                                out=U[:, j, :], in_=U[:, j, :],
                                compare_op=ALU.is_ge, fill=0.0,
                                base=0, channel_multiplier=-1,
                                pattern=[[1, P]])
                    return U, blocks

                def avs(t, s, bi, U, blocks):
                    kv = 2 * (t // 4) + s
                    if (t, s) not in psav:
                        psav[(t, s)] = pspool.tile([P, L], F32, tag="av",
                                                   name=f"av{t}_{s}")
                    for j, (qb, kb) in enumerate(blocks):
                        nc.tensor.matmul(
                            psav[(t, s)][:, qb * P:(qb + 1) * P],
                            _sb(Vst[kb], kv * P + (1 - s) * 64,
                                [[576, P], [1, P]]),
                            U[:, j, :],
                            start=(kb == 0), stop=(kb == qb))
                    if bi == NBATCH - 1:
                        epilogue(t, s)

                def epilogue(t, s):
                    no, so = s * 64, (1 - s) * 64
                    rec = recpool.tile([P, L], F32, tag="rec",
                                       name=f"rec{t}_{s}")
                    nc.vector.reciprocal(rec[so:so + 64, :],
                                         psav[(t, s)][so:so + 64, :])
                    rec2 = recpool.tile([P, L], F32, tag="rec2",
                                        name=f"rec2{t}_{s}")
                    nc.sync.dma_start(rec2[no:no + 64, :],
                                      rec[so:so + 64, :])
                    nc.vector.tensor_mul(AT[t][no:no + 64, :],
                                         psav[(t, s)][no:no + 64, :],
                                         rec2[no:no + 64, :])

                # flat software pipeline: AVs lag scores by 2 batch-pairs
                fifo = []
                for t in range(8):
                    for bi in range(NBATCH):
                        for s in range(2):
                            fifo.append((t, s, bi) + scores(t, s, bi))
                        while len(fifo) > 4:
                            avs(*fifo.pop(0))
                    # drain stream A's tail so its epilogue overlaps the
                    # next tile's scores; B's last AV follows one pair in
                    while len(fifo) > (1 if t < 7 else 0):
                        avs(*fifo.pop(0))

        # ---------------- phase 3: out projection ------------------------
        with tc.tile_pool(name="osb", bufs=2) as osb, \
             tc.tile_pool(name="pso", bufs=1, space="PSUM") as pso:
            def p3_mm(ps, oc, b_, ic):
                nc.tensor.matmul(
                    ps[:], AT[ic][:, b_ * P:(b_ + 1) * P],
                    WO[ic // 4][:, (ic % 4) * D + oc * 512:
                                (ic % 4) * D + (oc + 1) * 512],
                    start=(ic == 0), stop=(ic == 7))

            for oc in range(4):
                ob_t = osb.tile([P, LB, 512], BF16, tag="ot", name=f"ot{oc}")
                # 4 psum banks (the ones freed before the last epilogues);
                # for oc 0 defer ic=7 (needs the last head's AT) past a full
                # wave of ic 0..6 so the PE never waits on the epilogue tail
                for w0, wn in ((0, 4), (4, 3)):
                    ps_w = []
                    for b_ in range(w0, w0 + wn):
                        ps = pso.tile([P, 512], F32, tag=f"pso{b_ % 4}",
                                      name=f"pso{oc}_{b_}")
                        ps_w.append(ps)
                        n_ic = 7 if oc == 0 else 8
                        for ic in range(n_ic):
                            p3_mm(ps, oc, b_, ic)
                        if oc != 0:
                            if oc == 3 and b_ >= 4:
                                nc.vector.tensor_copy(ob_t[:, b_, :], ps[:])
                            else:
                                nc.scalar.copy(ob_t[:, b_, :], ps[:])
                    if oc == 0:
                        for j, b_ in enumerate(range(w0, w0 + wn)):
                            p3_mm(ps_w[j], oc, b_, 7)
                            nc.scalar.copy(ob_t[:, b_, :], ps_w[j][:])
                if oc < 3:
                    nc.sync.dma_start(
                        _ap3(y, oc * 512, [[D, P], [P * D, LB], [1, 512]]),
                        ob_t[:])
                else:
                    nc.sync.dma_start(
                        _ap3(y, oc * 512, [[D, P], [P * D, 4], [1, 512]]),
                        ob_t[:, 0:4, :])
                    nc.sync.dma_start(
                        _ap3(y, oc * 512 + 4 * P * D,
                             [[D, P], [P * D, 3], [1, 512]]),
                        ob_t[:, 4:7, :])


# ---------------------------------------------------------------- host side
ROPE_BASE = 10000.0
_ROPE_PERM = np.concatenate([
    np.arange(0, 32, 2), np.arange(1, 32, 2),
    np.arange(32, 64, 2), np.arange(33, 64, 2)])
# local q head at (tile t, slot s) = _LOCAL_HEADS[2*t + s]
_LOCAL_HEADS = [8 * (t // 4) + 4 * s + t % 4 for t in range(8) for s in range(2)]


F8NP = (ml_dtypes.float8_e4m3fn if hasattr(ml_dtypes, "float8_e4m3fn")
        else ml_dtypes.float8_e4m3)


def _split8(a, scale):
    """hi/lo fp8 split at a single scale: a ~= (hi + lo)/scale."""
    hi = np.asarray(a * scale, dtype=F8NP)
    lo = np.asarray(a * scale - hi.astype(np.float32), dtype=F8NP)
    return np.ascontiguousarray(hi), np.ascontiguousarray(lo)


def _cos_sin_tables(temporal_pos, structural_pos):
    inv = (1.0 / ROPE_BASE) ** (np.arange(16, dtype=np.float64) / 16.0)
    tabs = {}
    for name, pos in (("t", temporal_pos), ("s", structural_pos)):
        ang = np.outer(inv, np.asarray(pos, dtype=np.float64))  # [16, L]
        tabs[name] = (np.cos(ang), np.sin(ang))
    ct, st = tabs["t"]
    cs, ss = tabs["s"]
    # 1/(SX*SW) folds the fp8 psum scale out of the roped q/k
    C64 = np.concatenate([ct, ct, cs, cs], axis=0) / (SX * SW)
    S64 = np.concatenate([-st, st, -ss, ss], axis=0) / (SX * SW)
    C = np.tile(C64, (2, 1)).astype(ml_dtypes.bfloat16)
    S = np.tile(S64, (2, 1)).astype(ml_dtypes.bfloat16)
    return np.ascontiguousarray(C), np.ascontiguousarray(S)


def make_in_maps(x, wq, wk, wv, wo, temporal_pos, structural_pos):
    bf = ml_dtypes.bfloat16
    x = np.asarray(x, dtype=np.float32)
    wq4 = np.asarray(wq, dtype=np.float32).reshape(HQ, HD, D)
    wk4 = np.asarray(wk, dtype=np.float32).reshape(HKV, HD, D)
    wv4 = np.asarray(wv, dtype=np.float32).reshape(HKV, HD, D)
    woT = np.asarray(wo, dtype=np.float32).T  # [D(in head dims), D(out)]
    C, S = _cos_sin_tables(temporal_pos, structural_pos)
    SWAP = np.zeros((P, P), dtype=ml_dtypes.bfloat16)
    for i in range(P):
        SWAP[i, i ^ 64] = 1.0

    in_maps = []
    for c in range(NCORES):
        b, g = divmod(c, 2)
        heads = [16 * g + h for h in _LOCAL_HEADS]
        wq_g = wq4[heads][:, _ROPE_PERM, :].reshape(GO, D)
        wk_g = wk4[4 * g:4 * g + 4][:, _ROPE_PERM, :].reshape(KVO, D)
        wv_g = wv4[4 * g:4 * g + 4].reshape(KVO, D)
        woT_g = np.concatenate([woT[64 * h:64 * h + 64, :] for h in heads])
        m = {"woT": np.ascontiguousarray(woT_g).astype(bf), "C": C, "S": S,
             "SW": SWAP}
        m["xTh"], m["xTl"] = _split8(np.ascontiguousarray(x[b].T), SX)
        m["wqTh"], m["wqTl"] = _split8(np.ascontiguousarray(wq_g.T), SW)
        m["wkTh"], m["wkTl"] = _split8(np.ascontiguousarray(wk_g.T), SW)
        m["wvTh"], m["wvTl"] = _split8(np.ascontiguousarray(wv_g.T), SW)
        in_maps.append(m)
    return in_maps


def kernel(x, wq, wk, wv, wo, temporal_pos, structural_pos, _trace=False):
    nc = build_nc()
    in_maps = make_in_maps(x, wq, wk, wv, wo, temporal_pos, structural_pos)
    res = bass_utils.run_bass_kernel_spmd(
        nc, in_maps, core_ids=list(range(NCORES)), trace=_trace)
    out = np.stack([
        np.asarray(res.results[2 * b]["y"], dtype=np.float32)
        + np.asarray(res.results[2 * b + 1]["y"], dtype=np.float32)
        for b in range(B)])
    kernel.last_result = res
    return out
